# revision 3
# baseline (speedup 1.0000x reference)
"""Bahdanau attention Trainium2 kernel.

Reference computation (per batch row b):
    dec_att = dec_out @ W_dec.T                               (B, ATT)
    scores[b,s] = att_v . tanh(enc_att[s,b,:] + dec_att[b,:])  (B, SEQ)
    weights = softmax(scores, axis=-1)                         (B, SEQ)
    context[b,:] = sum_s weights[b,s] * enc_outs[s,b,:]        (B, ENC_H)

Sharding: data-parallel over batch, 8 batches per core (64 / 8 cores).
Each core streams its (SEQ, 8, ATT) enc_att shard and (SEQ, 8, ENC_H)
enc_outs shard; W_dec (host-pretransposed) and att_v are replicated.

Per-core plan:
  phase 0: dec_attT[a, b] = sum_dh W_decT[dh, a] * dec_outT[dh, b]  (PE)
  phase 1: for each (batch, 512-seq group): DMA (128, 4x512) tile of
           enc_att (seq on partitions), PE-transpose 128x128 blocks into
           PSUM (att on partitions), ACT computes tanh(x + dec_attT[:,b])
           via the per-partition bias operand (PSUM -> SBUF, bf16), PE
           reduces against att_v (lhsT (128,1)) accumulating scores in
           PSUM.  Scores go through a DRAM tile so the softmax can run
           with batch on partitions.
  softmax: (8, SEQ) tile: reduce_max, ACT Exp(bias=-max, accum_out=den),
           reciprocal, scale.
  phase 2: weights gathered back as (128, SEQ/128) per batch (seq on
           partitions), cast bf16; stream enc_outs tiles, cast bf16, PE
           matvec (lhsT = weight column (128,1)) accumulating context in
           PSUM over all seq chunks.
"""

import numpy as np

import concourse.bass as bass
import concourse.tile as tile
from concourse import mybir
from concourse.bass_utils import run_bass_kernel_spmd
from concourse.tile import TileContext, ScopedClock
from concourse.masks import make_identity

SEQ, B, ENC_H, DEC_H, ATT = 2048, 64, 1024, 1024, 512
NCORES = 8
BS = B // NCORES

F32 = mybir.dt.float32
BF16 = mybir.dt.bfloat16
AXX = mybir.AxisListType.X
AF = mybir.ActivationFunctionType


def split_multi_waits(nc):
    """walrus in this container rejects >1 sync-wait on several instruction
    encodings (CTRL Drain, LDWEIGHTS, ...).  Hoist all but the last wait of
    any multi-wait instruction onto fresh single-wait NoOps placed directly
    before it on the same engine (same semantics: engines execute in order).
    """
    for f in nc.m.functions:
        for blk in f.blocks:
            insts = list(blk.instructions)
            out = []
            changed = False
            for inst in insts:
                si = inst.sync_info
                waits = list(si.on_wait) if si and si.on_wait else []
                if len(waits) > 1:
                    changed = True
                    for w in waits[:-1]:
                        nop = mybir.InstNoOp(
                            name=f"I-{nc.next_id()}", ins=[], outs=[])
                        nop.engine = inst.engine
                        nop.sync_info = mybir.SyncInfo(
                            on_wait=[w], on_update=[])
                        nc.register_instruction(nop, overwrite=True)
                        out.append(nop)
                    si.on_wait = waits[-1:]
                out.append(inst)
            if changed:
                blk.instructions = out


def build_program(seq=SEQ, bs=BS):
    nc = bass.Bass("TRN2", target_bir_lowering=False, debug=False,
                   num_devices=NCORES)
    enc_att = nc.dram_tensor("enc_att", [seq, bs, ATT], F32,
                             kind="ExternalInput")
    enc_outs = nc.dram_tensor("enc_outs", [seq, bs, ENC_H], F32,
                              kind="ExternalInput")
    dec_outT = nc.dram_tensor("dec_outT", [DEC_H, bs], F32,
                              kind="ExternalInput")
    w_decT = nc.dram_tensor("w_decT", [DEC_H, ATT], F32,
                            kind="ExternalInput")
    att_v = nc.dram_tensor("att_v", [ATT], F32, kind="ExternalInput")
    ctx_out = nc.dram_tensor("context", [bs, ENC_H], F32,
                             kind="ExternalOutput")
    w_out = nc.dram_tensor("weights", [bs, seq], F32, kind="ExternalOutput")

    NA = ATT // 128          # 4 att chunks
    ND = DEC_H // 128        # 8 dec_h chunks
    NG1 = seq // 512         # seq groups (phase 1)
    NC1 = seq // 128         # seq chunks of 128

    with TileContext(nc) as tc:
        with tc.tile_pool(name="consts", bufs=1) as consts, \
             tc.tile_pool(name="dram", bufs=1, space="DRAM") as dram_pool, \
             tc.tile_pool(name="ea", bufs=3) as ea_pool, \
             tc.tile_pool(name="th", bufs=4) as th_pool, \
             tc.tile_pool(name="row", bufs=2) as row_pool, \
             tc.tile_pool(name="sm", bufs=1) as sm_pool, \
             tc.tile_pool(name="eo", bufs=3) as eo_pool, \
             tc.tile_pool(name="eob", bufs=4) as eob_pool, \
             tc.tile_pool(name="wc", bufs=2) as wc_pool, \
             tc.tile_pool(name="crow", bufs=2) as crow_pool:

            ident = consts.tile([128, 128], F32)
            make_identity(nc, ident)

            # ---- phase 0: dec_attT (att on partitions) ----
            wdt = consts.tile([128, ND, ATT], F32)
            nc.sync.dma_start(
                out=wdt, in_=w_decT[:, :].rearrange("(d k) a -> k d a", k=128))
            dot_t = consts.tile([128, ND, bs], F32)
            nc.sync.dma_start(
                out=dot_t,
                in_=dec_outT[:, :].rearrange("(d k) b -> k d b", k=128))
            v_sb = consts.tile([128, NA], F32)
            nc.sync.dma_start(
                out=v_sb, in_=att_v[:].rearrange("(c k) -> k c", k=128))
            v_bf = consts.tile([128, NA], BF16)
            nc.vector.tensor_copy(out=v_bf, in_=v_sb)

            datt = consts.tile([128, NA, bs], F32)
            with tc.tile_pool(name="ps0", bufs=2, space="PSUM") as ps0:
                for c in range(NA):
                    acc = ps0.tile([128, bs], F32)
                    for d in range(ND):
                        nc.tensor.matmul(
                            acc,
                            lhsT=wdt[:, d, c * 128:(c + 1) * 128],
                            rhs=dot_t[:, d, :],
                            start=(d == 0), stop=(d == ND - 1))
                    nc.scalar.copy(out=datt[:, c, :], in_=acc)

            # ---- phase 1: scores ----
            scd = dram_pool.tile([bs, seq], F32)
            with tc.tile_pool(name="tp", bufs=4, space="PSUM") as tp_ps, \
                 tc.tile_pool(name="sc", bufs=2, space="PSUM") as sc_ps:
                for j in range(bs):
                    row = row_pool.tile([1, seq], F32)
                    for g in range(NG1):
                        ea = ea_pool.tile([128, 4, ATT], F32)
                        nc.sync.dma_start(
                            out=ea,
                            in_=enc_att[g * 512:(g + 1) * 512, j, :]
                            .rearrange("(gg k) a -> k gg a", k=128))
                        sc = sc_ps.tile([1, 512], F32)
                        for c in range(NA):
                            tp = tp_ps.tile([128, 512], F32)
                            for k in range(4):
                                nc.tensor.transpose(
                                    tp[:, k * 128:(k + 1) * 128],
                                    ea[:, k, c * 128:(c + 1) * 128],
                                    ident)
                            th = th_pool.tile([128, 512], BF16)
                            nc.scalar.activation(
                                out=th, in_=tp, func=AF.Tanh,
                                bias=datt[:, c, j:j + 1], scale=1.0)
                            nc.tensor.matmul(
                                sc, lhsT=v_bf[:, c:c + 1], rhs=th,
                                start=(c == 0), stop=(c == NA - 1))
                        nc.vector.tensor_copy(
                            out=row[:, g * 512:(g + 1) * 512], in_=sc)
                    nc.sync.dma_start(out=scd[j:j + 1, :], in_=row)

            # ---- softmax over seq, batch on partitions ----
            sc_all = sm_pool.tile([bs, seq], F32)
            nc.sync.dma_start(out=sc_all, in_=scd[:, :])
            mx = sm_pool.tile([bs, 1], F32)
            nc.vector.reduce_max(out=mx, in_=sc_all, axis=AXX)
            nmx = sm_pool.tile([bs, 1], F32)
            nc.scalar.mul(out=nmx, in_=mx, mul=-1.0)
            den = sm_pool.tile([bs, 1], F32)
            wun = sm_pool.tile([bs, seq], F32)
            nc.scalar.activation(out=wun, in_=sc_all, func=AF.Exp,
                                 bias=nmx, scale=1.0, accum_out=den)
            inv = sm_pool.tile([bs, 1], F32)
            nc.vector.reciprocal(out=inv, in_=den)
            w_sb = sm_pool.tile([bs, seq], F32)
            nc.vector.tensor_scalar_mul(out=w_sb, in0=wun, scalar1=inv)
            nc.sync.dma_start(out=w_out[:, :], in_=w_sb)
            wd = dram_pool.tile([bs, seq], F32)
            nc.sync.dma_start(out=wd[:, :], in_=w_sb)

            # ---- phase 2: context ----
            with tc.tile_pool(name="cps", bufs=2, space="PSUM") as ctx_ps:
                for j in range(bs):
                    wc = wc_pool.tile([128, NC1], F32)
                    nc.sync.dma_start(
                        out=wc, in_=wd[j, :].rearrange("(i k) -> k i", k=128))
                    wcb = wc_pool.tile([128, NC1], BF16)
                    nc.vector.tensor_copy(out=wcb, in_=wc)
                    c0 = ctx_ps.tile([1, 512], F32)
                    c1 = ctx_ps.tile([1, 512], F32)
                    for g in range(NG1):
                        eo = eo_pool.tile([128, 4, ENC_H], F32)
                        nc.sync.dma_start(
                            out=eo,
                            in_=enc_outs[g * 512:(g + 1) * 512, j, :]
                            .rearrange("(gg k) h -> k gg h", k=128))
                        for k in range(4):
                            i = g * 4 + k
                            eb = eob_pool.tile([128, ENC_H], BF16)
                            nc.vector.tensor_copy(out=eb, in_=eo[:, k, :])
                            nc.tensor.matmul(
                                c0, lhsT=wcb[:, i:i + 1], rhs=eb[:, 0:512],
                                start=(i == 0), stop=(i == NC1 - 1))
                            nc.tensor.matmul(
                                c1, lhsT=wcb[:, i:i + 1], rhs=eb[:, 512:1024],
                                start=(i == 0), stop=(i == NC1 - 1))
                    crow = crow_pool.tile([1, ENC_H], F32)
                    nc.vector.tensor_copy(out=crow[:, 0:512], in_=c0)
                    nc.vector.tensor_copy(out=crow[:, 512:1024], in_=c1)
                    nc.sync.dma_start(out=ctx_out[j:j + 1, :], in_=crow)

    split_multi_waits(nc)
    return nc


def make_in_maps(dec_out, enc_outs, enc_att, W_dec, att_v):
    W_decT = np.ascontiguousarray(W_dec.T)
    in_maps = []
    for c in range(NCORES):
        sl = slice(c * BS, (c + 1) * BS)
        in_maps.append({
            "enc_att": np.ascontiguousarray(enc_att[:, sl, :]),
            "enc_outs": np.ascontiguousarray(enc_outs[:, sl, :]),
            "dec_outT": np.ascontiguousarray(dec_out[sl, :].T),
            "w_decT": W_decT,
            "att_v": np.asarray(att_v),
        })
    return in_maps


def kernel(dec_out, enc_outs, enc_att, W_dec, att_v):
    dec_out = np.asarray(dec_out)
    enc_outs = np.asarray(enc_outs)
    enc_att = np.asarray(enc_att)
    W_dec = np.asarray(W_dec)
    att_v = np.asarray(att_v)
    nc = build_program()
    in_maps = make_in_maps(dec_out, enc_outs, enc_att, W_dec, att_v)
    res = run_bass_kernel_spmd(nc, in_maps, list(range(NCORES)))
    context = np.concatenate(
        [res.results[i]["context"] for i in range(NCORES)], axis=0)
    weights = np.concatenate(
        [res.results[i]["weights"] for i in range(NCORES)], axis=0)
    return (context, weights)
